# revision 14
# baseline (speedup 1.0000x reference)
"""GCN2 (2-layer GCNII + avg-pool + MLP decoder) on 8 Trainium2 NeuronCores.

Strategy (per sharding hint): 1D node partition of the destination side.
Core c owns nodes [c*NPC, (c+1)*NPC). Edges are routed to the core owning
their dst. Per core, per layer:

  - edges are grouped by (dst window, src chunk, dst tile) with a static
    (max-over-cores) block structure so one SPMD program serves all cores
  - source rows are fetched with dma_gather (512B fp32 rows for layer 1 from
    the replicated feature table; 256B bf16 rows for layer 2 from AllGather'd
    norm-scaled x1 tables; int16 indices force <=32768-row chunk tables)
  - the segmented scatter-add is a PE matmul per 128-edge block:
    psum[feat, dst_tile] += G_block^T-style accumulation with a selection
    matrix S[e, d] = w_e * (dstlocal_e == d) built on DVE via iota-compare
  - self loops enter via norm^2 ⊙ featT streams (never gathered)
  - epilogue folds GCNII algebra into W1e = a(1-b)I + ab w1 style matrices;
    relu+bias on ACT; x1*norm is transposed back to node-major via PE and
    AllGather'd in 4 chunks to form the layer-2 gather tables
  - graph avg-pool via PE matmuls against a one-hot graph matrix, AllReduce,
    then the tiny MLP + sigmoid on every core.

Host-side work is strictly index/layout preprocessing (degree counts,
normalization constants, edge partition, padding, replication, transposes
of input tensors) -- all float compute on node features happens on device.
"""

import math
import numpy as np
from contextlib import ExitStack
from dataclasses import dataclass

ALPHA = 0.5
BETA1 = math.log(1.0 / 1 + 1)
BETA2 = math.log(1.0 / 2 + 1)


@dataclass
class Cfg:
    N: int = 100000
    NG: int = 64          # graphs
    D: int = 128
    PH: int = 32          # MLP hidden
    NC: int = 8           # cores
    DW: int = 500         # dst window width
    TILE: int = 250       # dst tile width (PSUM matmul N)
    CL1: int = 32768      # layer-1 chunk rows

    @property
    def NPC(self):
        return self.N // self.NC

    @property
    def NW(self):
        return self.NPC // self.DW

    @property
    def NT(self):
        return self.DW // self.TILE

    @property
    def L2C(self):
        return self.NPC // 4          # per-core rows per AllGather chunk

    @property
    def L2ROWS(self):
        return self.NC * self.L2C     # rows per layer-2 chunk table


def _chunk_l1(cfg, src):
    return src // cfg.CL1, (src % cfg.CL1).astype(np.int64)


def _chunk_l2(cfg, src):
    c2 = src // cfg.NPC
    r = src % cfg.NPC
    k = r // cfg.L2C
    loc = c2 * cfg.L2C + (r % cfg.L2C)
    return k, loc


def _layer_structure(cfg, dst_local_all, chunk_all, core_all, nch):
    """Static (max-over-core) block structure for one layer.

    Returns B[w][k][t] block counts plus derived offsets."""
    NW, NT, CH = cfg.NW, cfg.NT, nch
    nkeys = NW * CH * NT
    key = ((dst_local_all // cfg.DW) * CH + chunk_all) * NT + \
        ((dst_local_all % cfg.DW) // cfg.TILE)
    counts = np.zeros((cfg.NC, nkeys), np.int64)
    flat = core_all * nkeys + key
    bc = np.bincount(flat, minlength=cfg.NC * nkeys)
    counts = bc.reshape(cfg.NC, nkeys)
    cmax = counts.max(axis=0)
    B = np.ceil(cmax / 128).astype(np.int64)          # [nkeys]
    return B.reshape(NW, CH, NT)


def _pack_layer(cfg, B, dst_local, chunk, loc, w_e, nch):
    """Per-core packed idx/dstloc/w arrays for one layer."""
    NW, NT, CH = cfg.NW, cfg.NT, nch
    Bf = B.reshape(-1)                                 # (w,k,t) nesting
    slot_base = np.concatenate([[0], np.cumsum(Bf * 128)])[:-1]
    TOT = int(Bf.sum() * 128)
    key = ((dst_local // cfg.DW) * CH + chunk) * NT + \
        ((dst_local % cfg.DW) // cfg.TILE)
    order = np.argsort(key, kind="stable")
    ks = key[order]
    # rank within group
    grp_start = np.searchsorted(ks, np.arange(NW * CH * NT))
    rank = np.arange(len(ks)) - grp_start[ks]
    slot = slot_base[ks] + rank
    idxbuf = np.zeros(TOT, np.int16)
    dlbuf = np.full(TOT, 300.0, np.float32)
    wbuf = np.zeros(TOT, np.float32)
    idxbuf[slot] = loc[order].astype(np.int16)
    dlbuf[slot] = (dst_local[order] % cfg.TILE).astype(np.float32)
    wbuf[slot] = w_e[order]
    idx_dev = np.tile(idxbuf.reshape(-1, 16).T, (8, 1)).copy()
    # S matrix blocks ordered (w,t,k,b); slots are ordered (w,k,t)
    Bwkt = B.reshape(NW, CH, NT)
    blk_base_wkt = np.concatenate([[0], np.cumsum(Bf)])
    dl_by_block = dlbuf.reshape(-1, 128)          # [NBLK(w,k,t), 128]
    order_blocks = []
    for w in range(NW):
        for t in range(NT):
            for k in range(CH):
                key = (w * CH + k) * NT + t
                b0 = blk_base_wkt[key]
                order_blocks.extend(range(b0, b0 + Bf[key]))
    dl_wtkb = dl_by_block[np.array(order_blocks, np.int64)]  # [NBLK,128]
    import ml_dtypes
    smat = (dl_wtkb[:, :, None] ==
            np.arange(cfg.TILE, dtype=np.float32)[None, None, :])
    smat = smat.astype(ml_dtypes.bfloat16).transpose(1, 0, 2)  # [128,NBLK,T]
    smat = np.ascontiguousarray(smat.reshape(128, -1))
    return idx_dev, smat


def _build_structure(cfg, src, dst, graph_ids):
    """All static metadata + per-core host arrays."""
    src = np.asarray(src).astype(np.int64)
    dst = np.asarray(dst).astype(np.int64)
    graph_ids = np.asarray(graph_ids).astype(np.int64)
    N = cfg.N
    deg = np.bincount(dst, minlength=N).astype(np.float64) + 1.0
    norm = (1.0 / np.sqrt(deg)).astype(np.float32)

    core = dst // cfg.NPC
    dst_local = dst % cfg.NPC
    ch1, loc1 = _chunk_l2(cfg, src)
    ch2, loc2 = ch1, loc1

    B1 = _layer_structure(cfg, dst_local, ch1, core, 4)
    B2 = B1

    per_core = []
    for c in range(cfg.NC):
        m = core == c
        dl_c = dst_local[m]
        i1, smat = _pack_layer(cfg, B1, dl_c, ch1[m], loc1[m],
                               norm[src[m]], 4)
        per_core.append(dict(idx1=i1, smat=smat))

    cnt = np.bincount(graph_ids, minlength=cfg.NG).astype(np.float32)
    cntinv = (1.0 / np.maximum(cnt, 1.0)).astype(np.float32)
    return dict(B1=B1, B2=B2, norm=norm, cntinv=cntinv, per_core=per_core,
                graph_ids=graph_ids)


def _emit_layer(nc, tc, ctx, cfg, pools, consts, layer, B, tables, streams,
                sinks):
    """Emit one GCN2 layer for the Tile program."""
    import concourse.mybir as mybir

    NW, NT, CH = cfg.NW, cfg.NT, 4
    TILE, DW = cfg.TILE, cfg.DW
    qrr = [0]

    idx_dram, smat_dram = streams["idx"], streams["smat"]
    featT_dram, normb_dram, nsqb_dram = (streams["featT"], streams["normb"],
                                         streams["nsqb"])
    W1e, W2e, b_sb = consts[f"W1e{layer}"], consts[f"W2e{layer}"], \
        consts[f"b{layer}"]
    idbf = consts["idbf"]
    idf32 = consts["idf32"]

    gpool_bf, spool, ppool_agg, ppool_rst, ppool_tr, work, \
        stream_pool, idx_pool, tr_out = (
            pools["gbf"], pools["s"], pools["pagg"],
            pools["prst"], pools["ptr"], pools["work"], pools["stream"],
            pools["idx"], pools["trout"])

    Bw = B.reshape(NW, CH, NT)
    blk_in_win = Bw.reshape(NW, -1).sum(axis=1)         # blocks per window
    win_base = np.concatenate([[0], np.cumsum(blk_in_win)])

    n_tr = (DW + 127) // 128
    f32 = mybir.dt.float32
    bf16 = mybir.dt.bfloat16

    for w in range(NW):
        J = int(blk_in_win[w])
        if J == 0:
            continue
        base = int(win_base[w])
        nidx_w = J * 128
        # streamed metadata
        idxw = idx_pool.tile([128, nidx_w // 16], mybir.dt.int16, tag="idxw")
        nc.sync.dma_start(idxw[:], idx_dram.ap()[:, base * 8:
                                                 base * 8 + nidx_w // 16])
        featw = stream_pool.tile([128, DW], f32, tag="featw")
        nc.sync.dma_start(featw[:], featT_dram.ap()[:, w * DW:(w + 1) * DW])
        normw = stream_pool.tile([128, DW], f32, tag="normw")
        nc.sync.dma_start(normw[:], normb_dram.ap()[:, w * DW:(w + 1) * DW])
        nsqw = stream_pool.tile([128, DW], f32, tag="nsqw")
        nc.vector.tensor_tensor(out=nsqw[:], in0=normw[:], in1=normw[:],
                                op=mybir.AluOpType.mult)

        # gathers (one per chunk)
        gbf = gpool_bf.tile([128, J, 128], bf16, tag="gbf")
        off_blocks = 0
        for k in range(CH):
            nb = int(Bw[w, k, :].sum())
            if nb == 0:
                continue
            tgt = gbf
            nc.gpsimd.dma_gather(
                out_ap=tgt[:, off_blocks:off_blocks + nb, :],
                in_ap=tables[k],
                idxs_ap=idxw[:, off_blocks * 8:(off_blocks + nb) * 8],
                num_idxs=nb * 128,
                num_idxs_reg=nb * 128,
                elem_size=128,
                single_packet=False,
                queue_num=qrr[0] % 4,
            )
            qrr[0] += 1
            off_blocks += nb

        # aggregation matmuls per dst tile
        hT = work.tile([128, DW], f32, tag="hT")
        scol = {"v": int(np.array(
            [Bw[ww].sum() for ww in range(w)]).sum()) if w else 0}
        for t in range(NT):
            ps = ppool_agg.tile([128, TILE], f32, tag="pagg")
            mlist = []
            for k in range(CH):
                off = int(Bw[w, :k, :].sum())
                for b in range(int(Bw[w, k, t])):
                    mlist.append(off + (int(Bw[w, k, 0]) if t == 1 else 0) + b)
            Jt = len(mlist)
            if Jt:
                stile = spool.tile([128, Jt, TILE], bf16, tag="s")
                nc.sync.dma_start(
                    stile[:],
                    smat_dram.ap()[:, scol["v"] * TILE:
                                   (scol["v"] + Jt) * TILE]
                    .rearrange("p (j d) -> p j d", d=TILE))
                scol["v"] += Jt
            for i, blk in enumerate(mlist):
                nc.tensor.matmul(ps[:], gbf[:, blk, :], stile[:, i, :],
                                 start=(i == 0), stop=(i == len(mlist) - 1))
            if not mlist:
                nc.vector.memset(ps[:], 0.0)
            # hT_tile = psum * norm
            nc.vector.tensor_tensor(
                out=hT[:, t * TILE:(t + 1) * TILE], in0=ps[:],
                in1=normw[:, t * TILE:(t + 1) * TILE],
                op=mybir.AluOpType.mult)
        # self-loop: hT += featT * nsq
        tmp2 = work.tile([128, DW], f32, tag="tmp2")
        nc.vector.tensor_tensor(out=tmp2[:], in0=featw[:], in1=nsqw[:],
                                op=mybir.AluOpType.mult)
        nc.vector.tensor_tensor(out=hT[:], in0=hT[:], in1=tmp2[:],
                                op=mybir.AluOpType.add)
        # epilogue: rst = W1e^T-style + W2e on feat0
        rst = ppool_rst.tile([128, DW], f32, tag="prst")
        nc.tensor.matmul(rst[:], W1e[:], hT[:], start=True, stop=False)
        nc.tensor.matmul(rst[:], W2e[:], featw[:], start=False, stop=True)
        xT = work.tile([128, DW], f32, tag="xT")
        nc.scalar.activation(xT[:], rst[:],
                             mybir.ActivationFunctionType.Relu, bias=b_sb[:])

        if layer == 1:
            x1s_stage = sinks["x1s_stage"]
            x1sT = work.tile([128, DW], bf16, tag="x1sT")
            nc.vector.tensor_tensor(out=x1sT[:], in0=xT[:], in1=normw[:],
                                    op=mybir.AluOpType.mult)
            for c4 in range(n_tr):
                cw = min(128, DW - c4 * 128)
                ptr = ppool_tr.tile([cw, 128], bf16, tag="ptr")
                nc.tensor.transpose(ptr[:], x1sT[:, c4 * 128:c4 * 128 + cw],
                                    idbf[:])
                trt = tr_out.tile([cw, 128], bf16, tag="trout")
                nc.vector.tensor_copy(trt[:], ptr[:])
                nc.sync.dma_start(
                    x1s_stage.ap()[w * DW + c4 * 128:
                                   w * DW + c4 * 128 + cw, :], trt[:])
            # chunked AllGather triggers
            for kk, wtrig in enumerate(sinks["ag_trigger"]):
                if w == wtrig:
                    L2C = cfg.L2C
                    nc.gpsimd.collective_compute(
                        "AllGather", mybir.AluOpType.bypass,
                        replica_groups=[list(range(cfg.NC))],
                        ins=[x1s_stage.ap()[kk * L2C:(kk + 1) * L2C, :].opt()],
                        outs=[sinks["ag_out"][kk].ap().opt()])
        else:
            pool_ps = sinks["pool_psum"]
            grone = sinks["grone"]
            for c4 in range(n_tr):
                cw = min(128, DW - c4 * 128)
                ptr = ppool_tr.tile([cw, 128], f32, tag="ptr")
                nc.tensor.transpose(ptr[:], xT[:, c4 * 128:c4 * 128 + cw],
                                    idf32[:])
                trt = tr_out.tile([cw, 128], f32, tag="troutf")
                nc.vector.tensor_copy(trt[:], ptr[:])
                grt = stream_pool.tile([cw, cfg.NG], f32, tag="grt")
                nc.sync.dma_start(
                    grt[:], grone.ap()[w * DW + c4 * 128:
                                       w * DW + c4 * 128 + cw, :])
                nc.tensor.matmul(pool_ps[:], trt[:], grt[:],
                                 start=(w == 0 and c4 == 0),
                                 stop=(w == NW - 1 and c4 == n_tr - 1))


def build_nc(cfg, B1, B2):
    import concourse.bass as bass  # noqa: F401
    import concourse.tile as tile
    from concourse import bacc, mybir

    f32 = mybir.dt.float32
    bf16 = mybir.dt.bfloat16
    i16 = mybir.dt.int16

    nc = bacc.Bacc("TRN2", debug=False, num_devices=cfg.NC,
                   dynamic_dma_scratch_size=16384, num_swdge_queues=4)

    NB1 = int(B1.sum())

    # inputs
    featrows = nc.dram_tensor("featrows", [cfg.NPC, 128], f32,
                              kind="ExternalInput")
    featT = nc.dram_tensor("featT", [128, cfg.NPC], f32, kind="ExternalInput")
    normb = nc.dram_tensor("normb", [128, cfg.NPC], f32, kind="ExternalInput")
    nsqb = nc.dram_tensor("nsqb", [128, cfg.NPC], f32, kind="ExternalInput")
    idx1 = nc.dram_tensor("idx1", [128, NB1 * 8], i16, kind="ExternalInput")
    smat_in = nc.dram_tensor("smat", [128, NB1 * cfg.TILE], bf16,
                             kind="ExternalInput")
    normwrap_in = nc.dram_tensor("normwrap",
                                 [128, (cfg.NPC + 127) // 128], f32,
                                 kind="ExternalInput")
    ident = nc.dram_tensor("ident", [128, 128], f32, kind="ExternalInput")
    w11 = nc.dram_tensor("w1_1", [128, 128], f32, kind="ExternalInput")
    w21 = nc.dram_tensor("w2_1", [128, 128], f32, kind="ExternalInput")
    w12 = nc.dram_tensor("w1_2", [128, 128], f32, kind="ExternalInput")
    w22 = nc.dram_tensor("w2_2", [128, 128], f32, kind="ExternalInput")
    b1_in = nc.dram_tensor("b_1", [128, 1], f32, kind="ExternalInput")
    b2_in = nc.dram_tensor("b_2", [128, 1], f32, kind="ExternalInput")
    dec1w_in = nc.dram_tensor("dec1w", [128, cfg.PH], f32,
                              kind="ExternalInput")
    dec1bb_in = nc.dram_tensor("dec1bb", [cfg.NG, cfg.PH], f32,
                               kind="ExternalInput")
    dec2wb_in = nc.dram_tensor("dec2wb", [cfg.NG, cfg.PH], f32,
                               kind="ExternalInput")
    dec2bb_in = nc.dram_tensor("dec2bb", [cfg.NG, 1], f32,
                               kind="ExternalInput")
    cntinv_in = nc.dram_tensor("cntinv", [128, cfg.NG], f32,
                               kind="ExternalInput")
    grone = nc.dram_tensor("grone", [cfg.NPC, cfg.NG], f32,
                           kind="ExternalInput")
    out = nc.dram_tensor("out", [cfg.NG, 1], f32, kind="ExternalOutput")

    # internal dram
    t1_stage = nc.dram_tensor("t1_stage", [cfg.NPC, 128], bf16)
    t1_ag = [nc.dram_tensor(f"t1ag{k}", [cfg.L2ROWS, 128], bf16,
                            addr_space="Shared") for k in range(4)]
    x1s_stage = nc.dram_tensor("x1s_stage", [cfg.NPC, 128], bf16)
    ag_out = [nc.dram_tensor(f"ag{k}", [cfg.L2ROWS, 128], bf16,
                             addr_space="Shared") for k in range(4)]
    ar_in = nc.dram_tensor("ar_in", [128, cfg.NG], f32)
    ar_out = nc.dram_tensor("ar_out", [128, cfg.NG], f32, addr_space="Shared")

    ag_trigger = [min(cfg.NW - 1,
                      int(np.ceil(cfg.L2C * (k + 1) / cfg.DW)) - 1)
                  for k in range(4)]

    with tile.TileContext(nc) as tc, ExitStack() as ctx:
        cpool = ctx.enter_context(tc.tile_pool(name="consts", bufs=1))
        pools = dict(
            boot=ctx.enter_context(tc.tile_pool(name="boot", bufs=1)),
            gbf=ctx.enter_context(tc.tile_pool(name="gbf", bufs=3)),
            s=ctx.enter_context(tc.tile_pool(name="s", bufs=3)),
            pagg=ctx.enter_context(
                tc.tile_pool(name="pagg", bufs=3, space="PSUM")),
            prst=ctx.enter_context(
                tc.tile_pool(name="prst", bufs=2, space="PSUM")),
            ptr=ctx.enter_context(
                tc.tile_pool(name="ptr", bufs=2, space="PSUM")),
            ppool=ctx.enter_context(
                tc.tile_pool(name="ppool", bufs=1, space="PSUM")),
            work=ctx.enter_context(tc.tile_pool(name="work", bufs=3)),
            stream=ctx.enter_context(tc.tile_pool(name="stream", bufs=3)),
            idx=ctx.enter_context(tc.tile_pool(name="idx", bufs=3)),
            trout=ctx.enter_context(tc.tile_pool(name="trout", bufs=3)),
        )
        f32_ = f32

        def load_const(name, dram, shape, dt=f32_):
            t = cpool.tile(shape, dt, tag=name)
            nc.sync.dma_start(t[:], dram.ap())
            return t

        idf32 = load_const("idf32", ident, [128, 128])
        nwrap_sb = load_const("normwrap", normwrap_in,
                              [128, (cfg.NPC + 127) // 128])
        idbf = cpool.tile([128, 128], bf16, tag="idbf")
        nc.vector.tensor_copy(idbf[:], idf32[:])
        b1_sb = load_const("b1", b1_in, [128, 1])
        b2_sb = load_const("b2", b2_in, [128, 1])
        dec1w_sb = load_const("dec1w", dec1w_in, [128, cfg.PH])
        dec1bb_sb = load_const("dec1bb", dec1bb_in, [cfg.NG, cfg.PH])
        dec2wb_sb = load_const("dec2wb", dec2wb_in, [cfg.NG, cfg.PH])
        dec2bb_sb = load_const("dec2bb", dec2bb_in, [cfg.NG, 1])
        cntinv_sb = load_const("cntinv", cntinv_in, [128, cfg.NG])

        consts = dict(idbf=idbf, idf32=idf32, b1=b1_sb, b2=b2_sb)
        # W effective matrices
        for lname, wdram_a, wdram_b, beta in (
                ("1", w11, w21, BETA1), ("2", w12, w22, BETA2)):
            for which, wd in (("W1e", wdram_a), ("W2e", wdram_b)):
                wsb = load_const(f"{which}{lname}_raw", wd, [128, 128])
                eff = cpool.tile([128, 128], f32_, tag=f"{which}{lname}")
                nc.vector.tensor_scalar_mul(eff[:], wsb[:],
                                            0.5 * beta)
                ih = cpool.tile([128, 128], f32_, tag=f"ih_{which}{lname}")
                nc.vector.tensor_scalar_mul(ih[:], idf32[:],
                                            0.5 * (1.0 - beta))
                nc.vector.tensor_add(eff[:], eff[:], ih[:])
                consts[f"{which}{lname}"] = eff

        pool_psum = pools["ppool"].tile([128, cfg.NG], f32_, tag="poolps")

        # startup: cast per-core feature rows to bf16, AllGather into the
        # 4 layer-1 gather chunk tables
        import concourse.mybir as mybir
        nj = cfg.NPC // 128
        rem = cfg.NPC - nj * 128
        BOOTC = 16
        for j0 in range(0, nj, BOOTC):
            nb = min(BOOTC, nj - j0)
            fr32 = pools["boot"].tile([128, nb, 128], f32_, tag="fr32")
            nc.sync.dma_start(
                fr32[:],
                featrows.ap()[j0 * 128:(j0 + nb) * 128, :]
                .rearrange("(j p) e -> p j e", p=128))
            frbf = pools["boot"].tile([128, nb, 128],
                                      mybir.dt.bfloat16, tag="frbf")
            nc.vector.tensor_tensor(
                out=frbf[:],
                in0=fr32[:],
                in1=nwrap_sb[:, j0:j0 + nb].broadcast_to((128, nb, 128)),
                op=mybir.AluOpType.mult)
            nc.sync.dma_start(
                t1_stage.ap()[j0 * 128:(j0 + nb) * 128, :]
                .rearrange("(j p) e -> p j e", p=128), frbf[:])
        if rem:
            ft32 = pools["boot"].tile([rem, 128], f32_, tag="fr32")
            nc.sync.dma_start(ft32[:], featrows.ap()[nj * 128:, :])
            ftbf = pools["boot"].tile([rem, 128], mybir.dt.bfloat16,
                                      tag="frbf")
            nc.vector.tensor_scalar(
                out=ftbf[:], in0=ft32[:],
                scalar1=nwrap_sb[0:rem, nj:nj + 1], scalar2=None,
                op0=mybir.AluOpType.mult)
            nc.sync.dma_start(t1_stage.ap()[nj * 128:, :], ftbf[:])
        for k in range(4):
            nc.gpsimd.collective_compute(
                "AllGather", mybir.AluOpType.bypass,
                replica_groups=[list(range(cfg.NC))],
                ins=[t1_stage.ap()[k * cfg.L2C:(k + 1) * cfg.L2C, :].opt()],
                outs=[t1_ag[k].ap().opt()])

        # layer 1
        ltab1 = [t1_ag[k].ap() for k in range(4)]
        _emit_layer(nc, tc, ctx, cfg, pools, consts, 1, B1, ltab1,
                    dict(idx=idx1, smat=smat_in, featT=featT,
                         normb=normb, nsqb=nsqb),
                    dict(x1s_stage=x1s_stage, ag_out=ag_out,
                         ag_trigger=ag_trigger))
        # layer 2
        ltab2 = [ag_out[k].ap() for k in range(4)]
        _emit_layer(nc, tc, ctx, cfg, pools, consts, 2, B2, ltab2,
                    dict(idx=idx1, smat=smat_in, featT=featT,
                         normb=normb, nsqb=nsqb),
                    dict(pool_psum=pool_psum, grone=grone))

        # pooled allreduce + MLP
        import concourse.mybir as mybir
        pooled_sb = cpool.tile([128, cfg.NG], f32_, tag="pooled")
        nc.vector.tensor_copy(pooled_sb[:], pool_psum[:])
        nc.sync.dma_start(ar_in.ap(), pooled_sb[:])
        nc.gpsimd.collective_compute(
            "AllReduce", mybir.AluOpType.add,
            replica_groups=[list(range(cfg.NC))],
            ins=[ar_in.ap().opt()], outs=[ar_out.ap().opt()])
        pooled2 = cpool.tile([128, cfg.NG], f32_, tag="pooled2")
        nc.sync.dma_start(pooled2[:], ar_out.ap())
        pmean = cpool.tile([128, cfg.NG], f32_, tag="pmean")
        nc.vector.tensor_tensor(out=pmean[:], in0=pooled2[:],
                                in1=cntinv_sb[:], op=mybir.AluOpType.mult)
        mlp_ps = pools["prst"].tile([cfg.NG, cfg.PH], f32_, tag="prst")
        nc.tensor.matmul(mlp_ps[:], pmean[:], dec1w_sb[:],
                         start=True, stop=True)
        h1 = cpool.tile([cfg.NG, cfg.PH], f32_, tag="h1")
        nc.vector.tensor_add(h1[:], mlp_ps[:], dec1bb_sb[:])
        nc.vector.tensor_scalar_max(h1[:], h1[:], 0.0)
        zt = cpool.tile([cfg.NG, cfg.PH], f32_, tag="zt")
        nc.vector.tensor_tensor(out=zt[:], in0=h1[:], in1=dec2wb_sb[:],
                                op=mybir.AluOpType.mult)
        z = cpool.tile([cfg.NG, 1], f32_, tag="z")
        nc.vector.reduce_sum(z[:], zt[:], axis=mybir.AxisListType.X)
        y = cpool.tile([cfg.NG, 1], f32_, tag="y")
        nc.scalar.activation(y[:], z[:],
                             mybir.ActivationFunctionType.Sigmoid,
                             bias=dec2bb_sb[:])
        nc.sync.dma_start(out.ap(), y[:])

    # Post-scheduling: pin each SWDGE gather's queue to its assigned DMASW
    # lane so a given Tile DMA semaphore only ever sees one queue.
    from concourse.tile_scheduler import PROC_NAMES
    import concourse.mybir as mybir_
    lane_of = {i: n for i, n in enumerate(PROC_NAMES)}
    for bb in nc.main_func.blocks:
        for ins in bb.instructions:
            if isinstance(ins, mybir_.InstDMAGatherAnt):
                proc = ins.bass_scheduled_proc
                name = lane_of.get(proc, "")
                if name.startswith("DMASW"):
                    ins.queue_num = int(name[5:]) % 4
    nc.compile()
    return nc


def _make_in_maps(cfg, meta, feature, w1_1, w2_1, b_1, w1_2, w2_2, b_2,
                  dec1_w, dec1_b, dec2_w, dec2_b):
    feature = np.ascontiguousarray(np.asarray(feature, np.float32))
    norm = meta["norm"]
    in_maps = []
    import ml_dtypes  # noqa: F401
    ident = np.eye(128, dtype=np.float32)
    dec1bb = np.tile(np.asarray(dec1_b, np.float32)[None, :], (cfg.NG, 1))
    dec2wb = np.tile(np.asarray(dec2_w, np.float32)[:, 0][None, :],
                     (cfg.NG, 1))
    dec2bb = np.full((cfg.NG, 1), np.float32(np.asarray(dec2_b)[0]))
    cntinv = np.tile(meta["cntinv"][None, :], (128, 1))
    gids = meta["graph_ids"]
    for c in range(cfg.NC):
        pc = meta["per_core"][c]
        sl = slice(c * cfg.NPC, (c + 1) * cfg.NPC)
        featT = np.ascontiguousarray(feature[sl].T)
        normb = np.tile(norm[sl][None, :], (128, 1))
        nsqb = normb * normb
        gr = np.zeros((cfg.NPC, cfg.NG), np.float32)
        gr[np.arange(cfg.NPC), gids[sl]] = 1.0
        ncols = (cfg.NPC + 127) // 128
        npad = ncols * 128 - cfg.NPC
        nwrap = np.concatenate([norm[sl], np.zeros(npad, np.float32)])
        nwrap = nwrap.reshape(-1, 128).T.copy()
        in_maps.append({
            "featrows": np.ascontiguousarray(feature[sl]),
            "featT": featT, "normb": normb,
            "nsqb": np.ascontiguousarray(nsqb),
            "idx1": pc["idx1"], "smat": pc["smat"], "normwrap": nwrap,
            "ident": ident,
            "w1_1": np.asarray(w1_1, np.float32),
            "w2_1": np.asarray(w2_1, np.float32),
            "w1_2": np.asarray(w1_2, np.float32),
            "w2_2": np.asarray(w2_2, np.float32),
            "b_1": np.asarray(b_1, np.float32)[:, None],
            "b_2": np.asarray(b_2, np.float32)[:, None],
            "dec1w": np.asarray(dec1_w, np.float32),
            "dec1bb": dec1bb, "dec2wb": dec2wb, "dec2bb": dec2bb,
            "cntinv": cntinv, "grone": gr,
        })
    return in_maps


_KERNEL_CACHE = {}


def _get_compiled(cfg, B1, B2):
    key = (tuple(cfg.__dict__.items()), B1.tobytes(), B2.tobytes())
    import hashlib
    key = hashlib.sha256(repr(key).encode()).hexdigest()
    if key not in _KERNEL_CACHE:
        _KERNEL_CACHE[key] = build_nc(cfg, B1, B2)
    return _KERNEL_CACHE[key]


def run(cfg, inputs, trace=False):
    from concourse.bass_utils import run_bass_kernel_spmd
    meta = _build_structure(cfg, inputs["src"], inputs["dst"],
                            inputs["graph_ids"])
    nc = _get_compiled(cfg, meta["B1"], meta["B2"])
    in_maps = _make_in_maps(
        cfg, meta, inputs["feature"], inputs["w1_1"], inputs["w2_1"],
        inputs["b_1"], inputs["w1_2"], inputs["w2_2"], inputs["b_2"],
        inputs["dec1_w"], inputs["dec1_b"], inputs["dec2_w"],
        inputs["dec2_b"])
    res = run_bass_kernel_spmd(nc, in_maps, list(range(cfg.NC)), trace=trace)
    return res.results[0]["out"].astype(np.float32), res


def kernel(**inputs):
    cfg = Cfg()
    out, _ = run(cfg, inputs, trace=False)
    return out
